# revision 27
# baseline (speedup 1.0000x reference)
"""Trainium2 Bass kernel for channel-wise EMA over per-step batch means.

Problem: x [4, 8192, 1024] f32, ema [1, 1024] f32 (initial state).
    m = mean(x, axis=0)                      # [S, D]
    e_s = a*e_{s-1} + (1-a)*m_s              # scan over S
    out = broadcast(e, [4, S, D])

Strategy: tensor-parallel over D (8 cores x 128 channels each). The EMA is a
linear recurrence computed with matmuls against constant decay operators:
  - per group of 4 chunks x 128 steps, 4 matmuls (one per batch entry)
    against LT4R (time-reversed lower-triangular decay / 4) accumulate the
    within-chunk EMA in PSUM [t', (c=4, d=128)], folding the batch mean
    into the contraction. Output rows are time-reversed within each chunk
    so each chunk's local-last lands in PSUM row 0 (32-aligned, readable
    by the vector engine); the host un-reverses for free.
  - cross-chunk carries follow carry[c] = a^128 * carry[c-1] + lasts[c-1],
    evaluated exactly as tiny fused scalar_tensor_tensor ops per group on
    the vector engine in flat [1, (c,d)] layout; the post-correction row 0
    of a group's last chunk is directly the next group's entry carry.
  - one rank-1 correction matmul (alpha powers x carries) accumulates into
    the group PSUM; one vector-engine evacuation per group, then DMA out.
  - x streams in as 0.5-2MB 3-dim DMAs per (batch, supergroup) on the SP
    hardware queue; outputs go out on the ACT queue. All matmul operands
    are float32r (fast PE streaming mode, ~tf32 precision).
"""

import numpy as np

ALPHA = 0.99
B, S, D = 4, 8192, 1024
N_CORES = 8
DSH = D // N_CORES        # 128 channels per core
T = 128                   # chunk length (matmul contraction)
G = 4                     # chunks per group
W = G * DSH               # 512 free width
NG = S // (T * G)         # 16 groups
ALPHA_T = float(np.float64(ALPHA) ** T)


def _consts():
    # Output rows are time-REVERSED within each chunk (out row t' holds
    # timestep 127-t'), so each chunk's local-last lands in PSUM row 0
    # (32-aligned, directly readable by the vector engine) and the
    # post-correction row 0 is exactly the next chunk's carry. The host
    # un-reverses with a free numpy reshuffle.
    al = np.float64(ALPHA)
    k = np.arange(T)[:, None]
    tp = np.arange(T)[None, :]
    t = (T - 1) - tp  # timestep held by output row t'
    # LT4R[k, t'] = 0.25*(1-a)*a^(t-k) for k <= t   (lhsT layout [K, M])
    lt4 = np.where(k <= t, 0.25 * (1.0 - al) * al ** (t - k), 0.0).astype(np.float32)
    # aTR[0, t'] = a^(t+1) = a^(128-t')
    at = (al ** (t[0].astype(np.float64) + 1)).astype(np.float32)[None, :]
    return lt4, at


def build_nc():
    import concourse.mybir as mybir
    import concourse.tile as tile
    from concourse import bacc
    from concourse.bass import ts as bts

    FP32 = mybir.dt.float32
    FP32R = mybir.dt.float32r
    MULT = mybir.AluOpType.mult
    ADD = mybir.AluOpType.add

    nc = bacc.Bacc(trn_type="TRN2")
    x_dram = nc.dram_tensor("x", [B, S, DSH], FP32R, kind="ExternalInput")
    e0_dram = nc.dram_tensor("ema", [1, DSH], FP32, kind="ExternalInput")
    out_dram = nc.dram_tensor("out", [S, DSH], FP32, kind="ExternalOutput")

    lt4_np, at_np = _consts()
    lt4_dram = nc.inline_tensor(lt4_np, "lt4c")
    at_dram = nc.inline_tensor(at_np, "atc")

    # DRAM views: s = c*128 + k globally; supergroups batch several groups
    # into one 3-dim DMA [k, c, d]. The final supergroups are single-group so
    # the pipeline tail after the last load stays short.
    SGS = [2] * 7 + [1, 1]
    assert sum(SGS) == NG
    xv = x_dram.rearrange("b (c k) d -> b k c d", k=T)
    ov = out_dram.rearrange("(g c k) d -> g k c d", g=NG, c=G, k=T)

    with tile.TileContext(nc) as tc:
        with (
            tc.tile_pool(name="const", bufs=1) as cpool,
            tc.tile_pool(name="xin", bufs=3) as xpool,
            tc.tile_pool(name="oout", bufs=6) as opool,
            tc.tile_pool(name="cflat", bufs=3) as fpool,
            tc.tile_pool(name="ypsum", bufs=6, space="PSUM") as ypool,
        ):
            lt4 = cpool.tile([T, T], FP32R)
            nc.scalar.dma_start(lt4[:], lt4_dram[:].bitcast(FP32R))
            at = cpool.tile([1, T], FP32R)
            nc.scalar.dma_start(at[:], at_dram[:].bitcast(FP32R))
            e0 = cpool.tile([1, DSH], FP32)
            nc.scalar.dma_start(e0[:], e0_dram[:])

            # per-group state emitted in a software-pipelined order so the
            # tensor engine is never head-of-line blocked by the carry chain
            state = {}

            def emit_load(sg, g0, ng):
                xts = []
                c0 = g0 * G
                for b in range(B):
                    xt = xpool.tile(
                        [T, ng * W], FP32R, name=f"x{sg}b{b}", tag=f"xt{b}"
                    )
                    nc.sync.dma_start(
                        xt.rearrange("k (c d) -> k c d", c=G * ng),
                        xv[b, :, c0 : c0 + G * ng, :],
                    )
                    xts.append(xt)
                for i in range(ng):
                    state[("x", g0 + i)] = (xts, i)

            def emit_front(g):
                xts, i = state.pop(("x", g))
                ypsum = ypool.tile([T, W], FP32, name=f"ypsum{g}", tag="yp")
                for b in range(B):
                    nc.tensor.matmul(
                        ypsum[:],
                        lt4[:],
                        xts[b][:, bts(i, W)],
                        start=(b == 0),
                        stop=(b == B - 1),
                    )
                state[g] = ypsum

            def emit_back(g):
                ypsum = state.pop(g)
                # carries, flat layout [1, (c,d)]:
                #   carry[4g] = post-correction row 0 of the previous group's
                #     last chunk (= E at group entry; e0 for g=0)
                #   carry[4g+c] = a^T * carry[4g+c-1] + pre-correction row 0
                #     of chunk c-1 (its local last)
                cflat = fpool.tile([1, W], FP32R, name=f"cf{g}", tag="cf")
                if g == 0:
                    nc.vector.tensor_copy(cflat[:, 0:DSH], e0[:])
                else:
                    prev_y = state["y_prev"]
                    nc.vector.tensor_copy(
                        cflat[:, 0:DSH], prev_y[0:1, bts(G - 1, DSH)]
                    )
                for c in range(1, G):
                    nc.vector.scalar_tensor_tensor(
                        cflat[:, bts(c, DSH)],
                        cflat[:, bts(c - 1, DSH)],
                        ALPHA_T,
                        ypsum[0:1, bts(c - 1, DSH)],
                        MULT,
                        ADD,
                    )
                state["y_prev"] = ypsum

                # correction: ypsum[t, (c,d)] += a^(t+1) * carry[c, d]
                nc.tensor.matmul(
                    ypsum[:],
                    at[:],
                    cflat[:],
                    start=False,
                    stop=True,
                    skip_group_check=True,
                )
                out_sb = opool.tile([T, W], FP32, name=f"os{g}", tag="os")
                nc.vector.tensor_copy(out_sb[:], ypsum[:])
                nc.scalar.dma_start(
                    ov[g], out_sb.rearrange("k (c d) -> k c d", c=G)
                )

            sg_start = {}
            g0 = 0
            for sg, ng in enumerate(SGS):
                sg_start[g0] = (sg, ng)
                g0 += ng
            for g in range(NG):
                if g in sg_start:
                    sg, ng = sg_start[g]
                    emit_load(sg, g, ng)
                emit_front(g)
                if g >= 1:
                    emit_back(g - 1)
            emit_back(NG - 1)

    nc.compile()
    return nc


_NC_CACHE = None


def _get_nc():
    global _NC_CACHE
    if _NC_CACHE is None:
        _NC_CACHE = build_nc()
    return _NC_CACHE


def run_device(x: np.ndarray, ema: np.ndarray, **kwargs):
    """Run on the 8 NeuronCores; returns (es [S, D], BassKernelResults)."""
    from concourse.bass_utils import run_bass_kernel_spmd

    x = np.ascontiguousarray(x, dtype=np.float32)
    ema = np.ascontiguousarray(ema, dtype=np.float32)
    nc = _get_nc()

    in_maps = []
    for core in range(N_CORES):
        sl = slice(core * DSH, (core + 1) * DSH)
        in_maps.append(
            {
                "x": np.ascontiguousarray(x[:, :, sl]),
                "ema": np.ascontiguousarray(ema[:, sl]),
            }
        )
    try:
        res = run_bass_kernel_spmd(
            nc, in_maps, core_ids=list(range(N_CORES)), **kwargs
        )
    except Exception:
        # transient device faults (e.g. NRT_EXEC_UNIT_UNRECOVERABLE after a
        # wedged prior run) typically clear on retry
        res = run_bass_kernel_spmd(
            nc, in_maps, core_ids=list(range(N_CORES)), **kwargs
        )
    # device output rows are time-reversed within each 128-step chunk
    es = np.concatenate(
        [
            res.results[i]["out"]
            .reshape(S // T, T, DSH)[:, ::-1, :]
            .reshape(S, DSH)
            for i in range(N_CORES)
        ],
        axis=1,
    )
    return es, res


def kernel(x: np.ndarray, ema: np.ndarray) -> np.ndarray:
    es, _ = run_device(x, ema)
    return np.ascontiguousarray(np.broadcast_to(es[None], (B, S, D)))


# revision 33
# speedup vs baseline: 1.0121x; 1.0121x over previous
"""Trainium2 Bass kernel for channel-wise EMA over per-step batch means.

Problem: x [4, 8192, 1024] f32, ema [1, 1024] f32 (initial state).
    m = mean(x, axis=0)                      # [S, D]
    e_s = a*e_{s-1} + (1-a)*m_s              # scan over S
    out = broadcast(e, [4, S, D])

Strategy: tensor-parallel over D (8 cores x 128 channels each). The EMA is a
linear recurrence computed with matmuls against constant decay operators:
  - per group of 4 chunks x 128 steps, 4 matmuls (one per batch entry)
    against LT4R (time-reversed lower-triangular decay / 4) accumulate the
    within-chunk EMA in PSUM [t', (c=4, d=128)], folding the batch mean
    into the contraction. Output rows are time-reversed within each chunk
    so each chunk's local-last lands in PSUM row 0 (32-aligned, readable
    by the vector engine); the host un-reverses for free.
  - cross-chunk carries follow carry[c] = a^128 * carry[c-1] + lasts[c-1],
    evaluated exactly as tiny fused scalar_tensor_tensor ops in flat
    [1, (c,d)] layout; each group computes the NEXT group's entry carry
    from pre-correction values before its own correction matmul, so the
    chain lives entirely on the vector engine and never waits on the PE.
  - one rank-1 correction matmul (alpha powers x carries) accumulates into
    the group PSUM; one vector-engine evacuation per group, then DMA out.
  - x streams in as 0.5-2MB 3-dim DMAs per (batch, supergroup) on the SP
    hardware queue; outputs go out on the ACT queue. All matmul operands
    are float32r (fast PE streaming mode, ~tf32 precision).
"""

import numpy as np

ALPHA = 0.99
B, S, D = 4, 8192, 1024
N_CORES = 8
DSH = D // N_CORES        # 128 channels per core
T = 128                   # chunk length (matmul contraction)
G = 4                     # chunks per group
W = G * DSH               # 512 free width
NG = S // (T * G)         # 16 groups
ALPHA_T = float(np.float64(ALPHA) ** T)


def _consts():
    # Output rows are time-REVERSED within each chunk (out row t' holds
    # timestep 127-t'), so each chunk's local-last lands in PSUM row 0
    # (32-aligned, directly readable by the vector engine) and the
    # post-correction row 0 is exactly the next chunk's carry. The host
    # un-reverses with a free numpy reshuffle.
    al = np.float64(ALPHA)
    k = np.arange(T)[:, None]
    tp = np.arange(T)[None, :]
    t = (T - 1) - tp  # timestep held by output row t'
    # LT4R[k, t'] = 0.25*(1-a)*a^(t-k) for k <= t   (lhsT layout [K, M])
    lt4 = np.where(k <= t, 0.25 * (1.0 - al) * al ** (t - k), 0.0).astype(np.float32)
    # aTR[0, t'] = a^(t+1) = a^(128-t')
    at = (al ** (t[0].astype(np.float64) + 1)).astype(np.float32)[None, :]
    return lt4, at


def build_nc():
    import concourse.mybir as mybir
    import concourse.tile as tile
    from concourse import bacc
    from concourse.bass import ts as bts

    FP32 = mybir.dt.float32
    FP32R = mybir.dt.float32r
    MULT = mybir.AluOpType.mult
    ADD = mybir.AluOpType.add

    nc = bacc.Bacc(trn_type="TRN2")
    x_dram = nc.dram_tensor("x", [B, S, DSH], FP32R, kind="ExternalInput")
    e0_dram = nc.dram_tensor("ema", [1, DSH], FP32, kind="ExternalInput")
    out_dram = nc.dram_tensor("out", [S, DSH], FP32, kind="ExternalOutput")

    lt4_np, at_np = _consts()
    lt4_dram = nc.inline_tensor(lt4_np, "lt4c")
    at_dram = nc.inline_tensor(at_np, "atc")

    # DRAM views: s = c*128 + k globally; supergroups batch several groups
    # into one 3-dim DMA [k, c, d]. The final NGF chunks are processed at
    # chunk granularity (NGF "fine" chunks) so the pipeline tail after the
    # last load stays short (per-chunk correction/evacuation/store).
    NGF = 4                      # fine (chunk-granular) tail chunks
    NGC = NG - NGF // G          # coarse groups (chunks 0 .. NG*G-NGF-1)
    SGS = [2] * 6 + [1, 1, 1]
    assert sum(SGS) == NGC and NGC * G + NGF == S // T
    xv = x_dram.rearrange("b (c k) d -> b k c d", k=T)
    ov = out_dram.rearrange("(g c k) d -> g k c d", g=NG, c=G, k=T)
    ovf = out_dram.rearrange("(cc k) d -> cc k d", k=T)

    with tile.TileContext(nc) as tc:
        with (
            tc.tile_pool(name="const", bufs=1) as cpool,
            tc.tile_pool(name="xin", bufs=3) as xpool,
            tc.tile_pool(name="oout", bufs=6) as opool,
            tc.tile_pool(name="cflat", bufs=3) as fpool,
            tc.tile_pool(name="ypsum", bufs=5, space="PSUM") as ypool,
            tc.tile_pool(name="ypsumf", bufs=3, space="PSUM") as ypoolf,
        ):
            lt4 = cpool.tile([T, T], FP32R)
            nc.scalar.dma_start(lt4[:], lt4_dram[:].bitcast(FP32R))
            at = cpool.tile([1, T], FP32R)
            nc.scalar.dma_start(at[:], at_dram[:].bitcast(FP32R))
            e0 = cpool.tile([1, DSH], FP32)
            nc.scalar.dma_start(e0[:], e0_dram[:])

            # per-group state emitted in a software-pipelined order so the
            # tensor engine is never head-of-line blocked by the carry chain
            state = {}

            def emit_load(sg, g0, ng):
                xts = []
                c0 = g0 * G
                for b in range(B):
                    xt = xpool.tile(
                        [T, ng * W], FP32R, name=f"x{sg}b{b}", tag=f"xt{b}"
                    )
                    nc.sync.dma_start(
                        xt.rearrange("k (c d) -> k c d", c=G * ng),
                        xv[b, :, c0 : c0 + G * ng, :],
                    )
                    xts.append(xt)
                for i in range(ng):
                    state[("x", g0 + i)] = (xts, i)

            def emit_front(g):
                xts, i = state.pop(("x", g))
                ypsum = ypool.tile([T, W], FP32, name=f"ypsum{g}", tag="yp")
                for b in range(B):
                    nc.tensor.matmul(
                        ypsum[:],
                        lt4[:],
                        xts[b][:, bts(i, W)],
                        start=(b == 0),
                        stop=(b == B - 1),
                    )
                state[g] = ypsum

            def emit_back(g):
                ypsum = state.pop(g)
                # carries, flat layout [1, (c,d)]:
                #   carry[4g+c] = a^T * carry[4g+c-1] + pre-correction row 0
                #     of chunk 4g+c-1 (its local last); carry[0] = e0.
                # The entry carry of group g+1 (and of the first fine chunk)
                # is computed HERE, before this group's correction matmul, so
                # the whole chain stays on the vector engine and never waits
                # for the tensor engine.
                if g == 0:
                    cflat = fpool.tile([1, W], FP32R, name="cf0", tag="cf")
                    nc.vector.tensor_copy(cflat[:, 0:DSH], e0[:])
                else:
                    cflat = state.pop("cf_next")
                for c in range(1, G):
                    nc.vector.scalar_tensor_tensor(
                        cflat[:, bts(c, DSH)],
                        cflat[:, bts(c - 1, DSH)],
                        ALPHA_T,
                        ypsum[0:1, bts(c - 1, DSH)],
                        MULT,
                        ADD,
                    )
                # entry carry for what follows (next coarse group or first
                # fine chunk), from PRE-correction row 0 of the last chunk
                if g + 1 < NGC:
                    nxt = fpool.tile([1, W], FP32R, name=f"cf{g+1}", tag="cf")
                    nc.vector.scalar_tensor_tensor(
                        nxt[:, 0:DSH],
                        cflat[:, bts(G - 1, DSH)],
                        ALPHA_T,
                        ypsum[0:1, bts(G - 1, DSH)],
                        MULT,
                        ADD,
                    )
                    state["cf_next"] = nxt
                else:
                    nxt = fpool.tile([1, DSH], FP32R, name="cfm_first", tag="cfm")
                    nc.vector.scalar_tensor_tensor(
                        nxt[:],
                        cflat[:, bts(G - 1, DSH)],
                        ALPHA_T,
                        ypsum[0:1, bts(G - 1, DSH)],
                        MULT,
                        ADD,
                    )
                    state["cfm_next"] = nxt

                # correction: ypsum[t, (c,d)] += a^(t+1) * carry[c, d]
                nc.tensor.matmul(
                    ypsum[:],
                    at[:],
                    cflat[:],
                    start=False,
                    stop=True,
                    skip_group_check=True,
                )
                out_sb = opool.tile([T, W], FP32, name=f"os{g}", tag="os")
                nc.vector.tensor_copy(out_sb[:], ypsum[:])
                nc.scalar.dma_start(
                    ov[g], out_sb.rearrange("k (c d) -> k c d", c=G)
                )

            # --- fine (chunk-granular) tail machinery ---
            CC0 = NGC * G  # first fine chunk index

            def emit_load_fine(h):
                # one load of 4 chunks per batch entry
                xts = []
                c0 = CC0 + 4 * h
                for b in range(B):
                    xt = xpool.tile(
                        [T, 4 * DSH], FP32R, name=f"xf{h}b{b}", tag=f"xt{b}"
                    )
                    nc.sync.dma_start(
                        xt.rearrange("k (c d) -> k c d", c=4),
                        xv[b, :, c0 : c0 + 4, :],
                    )
                    xts.append(xt)
                for i in range(4):
                    state[("xf", CC0 + 4 * h + i)] = (xts, i)

            def emit_front_fine(cc):
                xts, i = state.pop(("xf", cc))
                yp = ypoolf.tile([T, DSH], FP32, name=f"ypf{cc}", tag="ypf")
                for b in range(B):
                    nc.tensor.matmul(
                        yp[:],
                        lt4[:],
                        xts[b][:, bts(i, DSH)],
                        start=(b == 0),
                        stop=(b == B - 1),
                    )
                state[cc] = yp

            def emit_back_fine(cc):
                yp = state.pop(cc)
                cfm = state.pop("cfm_next")
                # next chunk's carry from PRE-correction row 0, before corr
                if cc + 1 < CC0 + NGF:
                    nxt = fpool.tile([1, DSH], FP32R, name=f"cfm{cc+1}", tag="cfm")
                    nc.vector.scalar_tensor_tensor(
                        nxt[:], cfm[:], ALPHA_T, yp[0:1, :], MULT, ADD
                    )
                    state["cfm_next"] = nxt
                nc.tensor.matmul(
                    yp[:],
                    at[:],
                    cfm[:],
                    start=False,
                    stop=True,
                    skip_group_check=True,
                )
                out_sb = opool.tile([T, DSH], FP32, name=f"osf{cc}", tag="osf")
                nc.vector.tensor_copy(out_sb[:], yp[:])
                nc.scalar.dma_start(ovf[cc], out_sb[:])

            sg_start = {}
            g0 = 0
            for sg, ng in enumerate(SGS):
                sg_start[g0] = (sg, ng)
                g0 += ng
            for g in range(NGC):
                if g in sg_start:
                    sg, ng = sg_start[g]
                    emit_load(sg, g, ng)
                emit_front(g)
                if g >= 1:
                    emit_back(g - 1)
            emit_back(NGC - 1)
            fines = list(range(CC0, CC0 + NGF))
            for idx, cc in enumerate(fines):
                if (cc - CC0) % 4 == 0:
                    emit_load_fine((cc - CC0) // 4)
                emit_front_fine(cc)
                if idx >= 1:
                    emit_back_fine(fines[idx - 1])
            emit_back_fine(fines[-1])

    nc.compile()
    return nc


_NC_CACHE = None


def _get_nc():
    global _NC_CACHE
    if _NC_CACHE is None:
        _NC_CACHE = build_nc()
    return _NC_CACHE


def run_device(x: np.ndarray, ema: np.ndarray, **kwargs):
    """Run on the 8 NeuronCores; returns (es [S, D], BassKernelResults)."""
    from concourse.bass_utils import run_bass_kernel_spmd

    x = np.ascontiguousarray(x, dtype=np.float32)
    ema = np.ascontiguousarray(ema, dtype=np.float32)
    nc = _get_nc()

    in_maps = []
    for core in range(N_CORES):
        sl = slice(core * DSH, (core + 1) * DSH)
        in_maps.append(
            {
                "x": np.ascontiguousarray(x[:, :, sl]),
                "ema": np.ascontiguousarray(ema[:, sl]),
            }
        )
    try:
        res = run_bass_kernel_spmd(
            nc, in_maps, core_ids=list(range(N_CORES)), **kwargs
        )
    except Exception:
        # transient device faults (e.g. NRT_EXEC_UNIT_UNRECOVERABLE after a
        # wedged prior run) typically clear on retry
        res = run_bass_kernel_spmd(
            nc, in_maps, core_ids=list(range(N_CORES)), **kwargs
        )
    # device output rows are time-reversed within each 128-step chunk
    es = np.concatenate(
        [
            res.results[i]["out"]
            .reshape(S // T, T, DSH)[:, ::-1, :]
            .reshape(S, DSH)
            for i in range(N_CORES)
        ],
        axis=1,
    )
    return es, res


def kernel(x: np.ndarray, ema: np.ndarray) -> np.ndarray:
    es, _ = run_device(x, ema)
    return np.ascontiguousarray(np.broadcast_to(es[None], (B, S, D)))


# revision 36
# speedup vs baseline: 1.0242x; 1.0120x over previous
"""Trainium2 Bass kernel for channel-wise EMA over per-step batch means.

Problem: x [4, 8192, 1024] f32, ema [1, 1024] f32 (initial state).
    m = mean(x, axis=0)                      # [S, D]
    e_s = a*e_{s-1} + (1-a)*m_s              # scan over S
    out = broadcast(e, [4, S, D])

Strategy: tensor-parallel over D (8 cores x 128 channels each). The EMA is a
linear recurrence computed with matmuls against constant decay operators:
  - per group of 4 chunks x 128 steps, 4 matmuls (one per batch entry)
    against LT4R (time-reversed lower-triangular decay / 4) accumulate the
    within-chunk EMA in PSUM [t', (c=4, d=128)], folding the batch mean
    into the contraction. Output rows are time-reversed within each chunk
    so each chunk's local-last lands in PSUM row 0 (32-aligned, readable
    by the vector engine); the host un-reverses for free.
  - cross-chunk carries follow carry[c] = a^128 * carry[c-1] + lasts[c-1],
    evaluated exactly as tiny fused scalar_tensor_tensor ops in flat
    [1, (c,d)] layout; each group computes the NEXT group's entry carry
    from pre-correction values before its own correction matmul, so the
    chain lives entirely on the vector engine and never waits on the PE.
  - one rank-1 correction matmul (alpha powers x carries) accumulates into
    the group PSUM; one vector-engine evacuation per group, then DMA out.
  - x streams in as 0.5-2MB 3-dim DMAs per (batch, supergroup) on the SP
    hardware queue; outputs go out on the ACT queue. All matmul operands
    are float32r (fast PE streaming mode, ~tf32 precision).
"""

import numpy as np

ALPHA = 0.99
B, S, D = 4, 8192, 1024
N_CORES = 8
DSH = D // N_CORES        # 128 channels per core
T = 128                   # chunk length (matmul contraction)
G = 4                     # chunks per group
W = G * DSH               # 512 free width
NG = S // (T * G)         # 16 groups
ALPHA_T = float(np.float64(ALPHA) ** T)


def _consts():
    # Output rows are time-REVERSED within each chunk (out row t' holds
    # timestep 127-t'), so each chunk's local-last lands in PSUM row 0
    # (32-aligned, directly readable by the vector engine) and the
    # post-correction row 0 is exactly the next chunk's carry. The host
    # un-reverses with a free numpy reshuffle.
    al = np.float64(ALPHA)
    k = np.arange(T)[:, None]
    tp = np.arange(T)[None, :]
    t = (T - 1) - tp  # timestep held by output row t'
    # LT4R[k, t'] = 0.25*(1-a)*a^(t-k) for k <= t   (lhsT layout [K, M])
    lt4 = np.where(k <= t, 0.25 * (1.0 - al) * al ** (t - k), 0.0).astype(np.float32)
    # aTR[0, t'] = a^(t+1) = a^(128-t')
    at = (al ** (t[0].astype(np.float64) + 1)).astype(np.float32)[None, :]
    return lt4, at


def build_nc():
    import concourse.mybir as mybir
    import concourse.tile as tile
    from concourse import bacc
    from concourse.bass import ts as bts

    FP32 = mybir.dt.float32
    FP32R = mybir.dt.float32r
    MULT = mybir.AluOpType.mult
    ADD = mybir.AluOpType.add

    nc = bacc.Bacc(trn_type="TRN2")
    x_dram = nc.dram_tensor("x", [B, S, DSH], FP32R, kind="ExternalInput")
    e0_dram = nc.dram_tensor("ema", [1, DSH], FP32, kind="ExternalInput")
    out_dram = nc.dram_tensor("out", [S, DSH], FP32, kind="ExternalOutput")

    lt4_np, at_np = _consts()
    lt4_dram = nc.inline_tensor(lt4_np, "lt4c")
    at_dram = nc.inline_tensor(at_np, "atc")

    # DRAM views: s = c*128 + k globally; supergroups batch several groups
    # into one 3-dim DMA [k, c, d]. The final NGF chunks are processed at
    # chunk granularity (NGF "fine" chunks) so the pipeline tail after the
    # last load stays short (per-chunk correction/evacuation/store).
    NGF = 4                      # fine (chunk-granular) tail chunks
    NGC = NG - NGF // G          # coarse groups (chunks 0 .. NG*G-NGF-1)
    SGS = [2] * 6 + [1, 1, 1]
    assert sum(SGS) == NGC and NGC * G + NGF == S // T
    xv = x_dram.rearrange("b (c k) d -> b k c d", k=T)
    ov = out_dram.rearrange("(g c k) d -> g k c d", g=NG, c=G, k=T)
    ovf = out_dram.rearrange("(pp c k) d -> pp k c d", c=2, k=T)

    with tile.TileContext(nc) as tc:
        with (
            tc.tile_pool(name="const", bufs=1) as cpool,
            tc.tile_pool(name="xin", bufs=3) as xpool,
            tc.tile_pool(name="oout", bufs=6) as opool,
            tc.tile_pool(name="cflat", bufs=3) as fpool,
            tc.tile_pool(name="ypsum", bufs=5, space="PSUM") as ypool,
            tc.tile_pool(name="ypsumf", bufs=3, space="PSUM") as ypoolf,
        ):
            lt4 = cpool.tile([T, T], FP32R)
            nc.scalar.dma_start(lt4[:], lt4_dram[:].bitcast(FP32R))
            at = cpool.tile([1, T], FP32R)
            nc.scalar.dma_start(at[:], at_dram[:].bitcast(FP32R))
            e0 = cpool.tile([1, DSH], FP32)
            nc.scalar.dma_start(e0[:], e0_dram[:])

            # per-group state emitted in a software-pipelined order so the
            # tensor engine is never head-of-line blocked by the carry chain
            state = {}

            def emit_load(sg, g0, ng):
                xts = []
                c0 = g0 * G
                for b in range(B):
                    xt = xpool.tile(
                        [T, ng * W], FP32R, name=f"x{sg}b{b}", tag=f"xt{b}"
                    )
                    nc.sync.dma_start(
                        xt.rearrange("k (c d) -> k c d", c=G * ng),
                        xv[b, :, c0 : c0 + G * ng, :],
                    )
                    xts.append(xt)
                for i in range(ng):
                    state[("x", g0 + i)] = (xts, i)

            def emit_front(g):
                xts, i = state.pop(("x", g))
                ypsum = ypool.tile([T, W], FP32, name=f"ypsum{g}", tag="yp")
                for b in range(B):
                    nc.tensor.matmul(
                        ypsum[:],
                        lt4[:],
                        xts[b][:, bts(i, W)],
                        start=(b == 0),
                        stop=(b == B - 1),
                    )
                state[g] = ypsum

            def emit_back(g):
                ypsum = state.pop(g)
                # carries, flat layout [1, (c,d)]:
                #   carry[4g+c] = a^T * carry[4g+c-1] + pre-correction row 0
                #     of chunk 4g+c-1 (its local last); carry[0] = e0.
                # The entry carry of group g+1 (and of the first fine chunk)
                # is computed HERE, before this group's correction matmul, so
                # the whole chain stays on the vector engine and never waits
                # for the tensor engine.
                if g == 0:
                    cflat = fpool.tile([1, W], FP32R, name="cf0", tag="cf")
                    nc.vector.tensor_copy(cflat[:, 0:DSH], e0[:])
                else:
                    cflat = state.pop("cf_next")
                for c in range(1, G):
                    nc.vector.scalar_tensor_tensor(
                        cflat[:, bts(c, DSH)],
                        cflat[:, bts(c - 1, DSH)],
                        ALPHA_T,
                        ypsum[0:1, bts(c - 1, DSH)],
                        MULT,
                        ADD,
                    )
                # entry carry for what follows (next coarse group or first
                # fine chunk), from PRE-correction row 0 of the last chunk
                if g + 1 < NGC:
                    nxt = fpool.tile([1, W], FP32R, name=f"cf{g+1}", tag="cf")
                    nc.vector.scalar_tensor_tensor(
                        nxt[:, 0:DSH],
                        cflat[:, bts(G - 1, DSH)],
                        ALPHA_T,
                        ypsum[0:1, bts(G - 1, DSH)],
                        MULT,
                        ADD,
                    )
                    state["cf_next"] = nxt
                else:
                    nxt = fpool.tile(
                        [1, 2 * DSH], FP32R, name="cfm_first", tag="cfm"
                    )
                    nc.vector.scalar_tensor_tensor(
                        nxt[:, bts(0, DSH)],
                        cflat[:, bts(G - 1, DSH)],
                        ALPHA_T,
                        ypsum[0:1, bts(G - 1, DSH)],
                        MULT,
                        ADD,
                    )
                    state["cfm_next"] = nxt

                # correction: ypsum[t, (c,d)] += a^(t+1) * carry[c, d]
                nc.tensor.matmul(
                    ypsum[:],
                    at[:],
                    cflat[:],
                    start=False,
                    stop=True,
                    skip_group_check=True,
                )
                out_sb = opool.tile([T, W], FP32, name=f"os{g}", tag="os")
                nc.vector.tensor_copy(out_sb[:], ypsum[:])
                nc.scalar.dma_start(
                    ov[g], out_sb.rearrange("k (c d) -> k c d", c=G)
                )

            # --- fine (pair-granular) tail machinery ---
            PP0 = NGC * G // 2  # first fine pair index
            NPF = NGF // 2

            def emit_load_fine(h):
                # one load of 4 chunks (2 pairs) per batch entry
                xts = []
                c0 = (PP0 + 2 * h) * 2
                for b in range(B):
                    xt = xpool.tile(
                        [T, 4 * DSH], FP32R, name=f"xf{h}b{b}", tag=f"xt{b}"
                    )
                    nc.sync.dma_start(
                        xt.rearrange("k (c d) -> k c d", c=4),
                        xv[b, :, c0 : c0 + 4, :],
                    )
                    xts.append(xt)
                for i in range(2):
                    state[("xf", PP0 + 2 * h + i)] = (xts, i)

            def emit_front_fine(pp):
                xts, i = state.pop(("xf", pp))
                yp = ypoolf.tile([T, 2 * DSH], FP32, name=f"ypf{pp}", tag="ypf")
                for b in range(B):
                    nc.tensor.matmul(
                        yp[:],
                        lt4[:],
                        xts[b][:, bts(i, 2 * DSH)],
                        start=(b == 0),
                        stop=(b == B - 1),
                    )
                state[pp] = yp

            def emit_back_fine(pp):
                yp = state.pop(pp)
                cfm = state.pop("cfm_next")  # [1, 2*DSH]; slice 0 filled
                # second chunk's carry within the pair (pre-correction row 0)
                nc.vector.scalar_tensor_tensor(
                    cfm[:, bts(1, DSH)],
                    cfm[:, bts(0, DSH)],
                    ALPHA_T,
                    yp[0:1, bts(0, DSH)],
                    MULT,
                    ADD,
                )
                # next pair's entry carry
                if pp + 1 < PP0 + NPF:
                    nxt = fpool.tile(
                        [1, 2 * DSH], FP32R, name=f"cfm{pp+1}", tag="cfm"
                    )
                    nc.vector.scalar_tensor_tensor(
                        nxt[:, bts(0, DSH)],
                        cfm[:, bts(1, DSH)],
                        ALPHA_T,
                        yp[0:1, bts(1, DSH)],
                        MULT,
                        ADD,
                    )
                    state["cfm_next"] = nxt
                nc.tensor.matmul(
                    yp[:],
                    at[:],
                    cfm[:],
                    start=False,
                    stop=True,
                    skip_group_check=True,
                )
                out_sb = opool.tile([T, 2 * DSH], FP32, name=f"osf{pp}", tag="osf")
                nc.vector.tensor_copy(out_sb[:], yp[:])
                nc.scalar.dma_start(
                    ovf[pp], out_sb.rearrange("k (c d) -> k c d", c=2)
                )

            sg_start = {}
            g0 = 0
            for sg, ng in enumerate(SGS):
                sg_start[g0] = (sg, ng)
                g0 += ng
            for g in range(NGC):
                if g in sg_start:
                    sg, ng = sg_start[g]
                    emit_load(sg, g, ng)
                emit_front(g)
                if g >= 1:
                    emit_back(g - 1)
            emit_back(NGC - 1)
            fines = list(range(PP0, PP0 + NPF))
            for idx, pp in enumerate(fines):
                if (pp - PP0) % 2 == 0:
                    emit_load_fine((pp - PP0) // 2)
                emit_front_fine(pp)
                if idx >= 1:
                    emit_back_fine(fines[idx - 1])
            emit_back_fine(fines[-1])

    nc.compile()
    return nc


_NC_CACHE = None


def _get_nc():
    global _NC_CACHE
    if _NC_CACHE is None:
        _NC_CACHE = build_nc()
    return _NC_CACHE


def run_device(x: np.ndarray, ema: np.ndarray, **kwargs):
    """Run on the 8 NeuronCores; returns (es [S, D], BassKernelResults)."""
    from concourse.bass_utils import run_bass_kernel_spmd

    x = np.ascontiguousarray(x, dtype=np.float32)
    ema = np.ascontiguousarray(ema, dtype=np.float32)
    nc = _get_nc()

    in_maps = []
    for core in range(N_CORES):
        sl = slice(core * DSH, (core + 1) * DSH)
        in_maps.append(
            {
                "x": np.ascontiguousarray(x[:, :, sl]),
                "ema": np.ascontiguousarray(ema[:, sl]),
            }
        )
    try:
        res = run_bass_kernel_spmd(
            nc, in_maps, core_ids=list(range(N_CORES)), **kwargs
        )
    except Exception:
        # transient device faults (e.g. NRT_EXEC_UNIT_UNRECOVERABLE after a
        # wedged prior run) typically clear on retry
        res = run_bass_kernel_spmd(
            nc, in_maps, core_ids=list(range(N_CORES)), **kwargs
        )
    # device output rows are time-reversed within each 128-step chunk
    es = np.concatenate(
        [
            res.results[i]["out"]
            .reshape(S // T, T, DSH)[:, ::-1, :]
            .reshape(S, DSH)
            for i in range(N_CORES)
        ],
        axis=1,
    )
    return es, res


def kernel(x: np.ndarray, ema: np.ndarray) -> np.ndarray:
    es, _ = run_device(x, ema)
    return np.ascontiguousarray(np.broadcast_to(es[None], (B, S, D)))
